# revision 23
# baseline (speedup 1.0000x reference)
"""Trainium2 Bass kernel: dense screened-Coulomb pair energy with periodic
minimum-image convention (N=6144 atoms, B=8 cells), row-summed per atom.

batch is sorted and cross-graph pairs are masked, so the N x N problem is
block-diagonal over the 8 graphs: one graph per NeuronCore.

Math (fractional coords, fs = frac - 0.5):
  f_k[i,j] = fs_k[j] - fs_k[i]            DVE tensor_scalar fp16
  r_k      = round(f_k)                   DVE magic-number round fp16
  y        = (f - r) @ C                  PE: 2 accumulating fp16 matmuls
                                          (block-diag cell stationary, 128-wide
                                          weights for fast-weight-load)
  sq       = y^2                          ACT Square / custom DVE op (split)
  q        = sum_k y_k^2                  PE: ones-blockdiag fp16 matmul
  kern     = exp(-sigma*r)/r, r=sqrt(q+soft^2)
           = exp(-0.5*t - sigma*exp(0.5*t + ln(sigma)))  with t = ln(q+soft^2)
                                          ACT Ln, ACT Exp, DVE stt, ACT Exp
                                          -- Ln/Exp/Square served by ONE table
                                          set (natural_log_exp_and_others): no
                                          ACT_TABLE_LOAD thrash, no reciprocal.
  acc[j]   = sum_i src_i * kern[i,j]      PE matvec, PSUM-resident accumulator
                                          (row sum == col sum by symmetry)
  host: E[j] = 0.5*src_j*acc_j - 0.5*src_j^2*exp(-sigma*soft)/soft

Atoms in groups of GA=42 (3 coord rows per atom = 126 of 128 partitions);
3 groups form a 126-atom macro. All tiles padded to 128 partitions; padded
weight columns are zero, so padded q/kern rows compute to harmless zeros and
are masked by zero entries in the src weights.
"""
import numpy as np

GA = 42            # atoms per row group
ROWS = 128         # partitions per tile (3*GA = 126 used)
GPM = 3            # groups per macro block
MACRO = GA * GPM   # 126 atoms per macro
MAGIC = 12582912.0  # 1.5 * 2**23: (x + MAGIC) - MAGIC == round(x) for |x| < 2**22
NCORES = 8
CHUNK = 512        # PSUM bank limit for one matmul output (fp32 values)

_cache = {}
_custom_ops = None


def _register_custom(name, spec):
    import concourse.dve_ops as dve_ops
    from concourse.dve_spec import lower, _has_src1
    from concourse.dve_uop import DveOpSpec

    opcode = dve_ops._CUSTOM_DVE_ROW_BASE + len(dve_ops.OPS)
    shas = {}
    for ver in ("v3", "v4"):
        tmp = DveOpSpec(name=name, opcode=opcode,
                        uops=lower(spec, ver=ver), rd1_en=_has_src1(spec))
        shas[ver] = tmp.sha(ver)
    op = dve_ops.DveOp(name, spec, subdim=False, uops_sha=shas)
    dve_ops.OPS.append(op)
    dve_ops.CUSTOM_DVE_SPECS[op.name] = op.spec
    dve_ops._SUB_OPCODE_FOR_NAME[op.name] = opcode
    return op


def _get_custom_ops():
    """Register (once) two custom DVE ops:
    PAIR_SQ: out = in0^2 (single source; PSUM-legal square for the DVE share
             of the y->sq work -- stock tensor_tensor(y, y) would need two
             PSUM streams, which the STT struct forbids).
    GWRAP:   out = f - round(f) with f = in0 + s0, round via the fp32
             magic-number trick. One op replaces the f and r tensor_scalars
             and halves the y matmuls (single stationary weight cb)."""
    global _custom_ops
    if _custom_ops is None:
        from concourse.dve_spec import Spec, Src0, Src1, C0, C1, sq, AluOp

        sq_spec = Spec(body=sq(Src0),
                       reference=lambda in0, in1, s0, s1, imm2: in0 * in0)
        f = Src0 + C0
        g_spec = Spec(body=f - ((f + C1) - C1),
                      reference=lambda in0, in1, s0, s1, imm2:
                      (in0 + s0) - (((in0 + s0) + s1) - s1))
        ms_spec = Spec(body=Src0 * Src1, accum=AluOp.ADD,
                       reference=lambda in0, in1, s0, s1, imm2: in0 * in1)
        _custom_ops = (_register_custom("PAIR_SQ", sq_spec),
                       _register_custom("GWRAP", g_spec),
                       _register_custom("MULSUM", ms_spec))
    return _custom_ops


def _pin_lnexp_table(nc, mybir):
    """Bias the activation-table-load pass so Ln and Exp both resolve to the
    natural_log_exp_and_others set (the only set containing both): drop Ln
    from natural_log and Exp from exp_and_others in the (cached) table dict.
    Square stays available in every set, so the whole kernel needs ONE
    ACT_TABLE_LOAD instead of two per macro."""
    from concourse.hw_specs import get_activation_tables
    tabs = get_activation_tables(nc.m.arch)
    AF = mybir.ActivationFunctionType
    if "natural_log_exp_and_others" in tabs:
        for name, fns in tabs.items():
            if name != "natural_log_exp_and_others":
                fns.discard(AF.Ln)
                fns.discard(AF.Exp)
                fns.discard(AF.Square)
                fns.discard(AF.Copy)


def _build(n_macros, n_groups, cols, sigma, soft):
    import concourse.bacc as bacc
    import concourse.mybir as mybir
    import concourse.tile as tile

    f32 = mybir.dt.float32
    f16 = mybir.dt.float16
    alu = mybir.AluOpType
    act = mybir.ActivationFunctionType
    pair_sq, gwrap, mulsum = _get_custom_ops()

    soft2 = float(np.float32(soft) * np.float32(soft))
    lnsig = float(np.log(np.float64(sigma)))
    chunks = [(c, min(cols, c + CHUNK)) for c in range(0, cols, CHUNK)]
    pw = -(-cols // CHUNK) * CHUNK

    nc = bacc.Bacc("TRN2", target_bir_lowering=False, debug=False)
    _pin_lnexp_table(nc, mybir)
    # const APs for activation biases, registered like the built-ins
    for name, val in (("c-soft2", soft2), ("c-lnsig", lnsig)):
        t = nc.alloc_sbuf_tensor(name, [128, 1], f32)
        nc.gpsimd.memset(t.ap(), val)
        nc.const_aps.aps[(f32, val)] = t.ap()

    blob_a_w = cols + 2 * n_groups          # f16 units: fb | negfa(f32)
    blob_b_w = ROWS + GPM * ROWS + n_macros + cols
    BLOBA = nc.declare_dram_parameter("BLOBA", [ROWS, blob_a_w], f16, isOutput=False)
    BLOBB = nc.declare_dram_parameter("BLOBB", [ROWS, blob_b_w], f16, isOutput=False)
    OUT = nc.declare_dram_parameter("OUT", [1, cols], f32, isOutput=True)
    OUTR = nc.declare_dram_parameter("OUTR", [ROWS, n_macros], f32, isOutput=True)

    # group g -> (macro, slot); last macro may have < GPM groups
    def macro_groups(m):
        return [g for g in range(GPM * m, min(GPM * (m + 1), n_groups))]

    with tile.TileContext(nc) as tc:
        with tc.tile_pool(name="const", bufs=1) as cpool, \
             tc.tile_pool(name="work", bufs=6) as pool, \
             tc.tile_pool(name="ypsum", bufs=2, space="PSUM") as ypool, \
             tc.tile_pool(name="qpsum", bufs=1, space="PSUM") as qpool, \
             tc.tile_pool(name="apsum", bufs=1, space="PSUM") as apool:
            bloba = cpool.tile([ROWS, blob_a_w], f16)
            blobb = cpool.tile([ROWS, blob_b_w], f16)
            rowacc = cpool.tile([ROWS, n_macros], f32)
            nc.sync.dma_start(bloba[:], BLOBA[:])
            nc.gpsimd.dma_start(blobb[:], BLOBB[:])
            fb = bloba[:, 0:cols]
            negfa = bloba[:, cols:cols + 2 * n_groups].bitcast(f32)
            cb = blobb[:, 0:ROWS]
            onesb = blobb[:, ROWS:ROWS + GPM * ROWS]
            srcst = blobb[:, ROWS + GPM * ROWS:ROWS + GPM * ROWS + n_macros]
            srcrep = blobb[:, ROWS + GPM * ROWS + n_macros:blob_b_w]

            acc = apool.tile([1, pw], f32)

            # triangle over macro-row blocks: block m covers row atoms of
            # macro m x global columns [126m, cols). Column sums accumulate
            # in PSUM via PE; row sums (columns beyond the own macro, to
            # avoid double-count) via tensor_tensor_reduce with a
            # partition-replicated src tile.
            bankA_stop = max(m for m in range(n_macros) if MACRO * m < CHUNK)
            for m in range(n_macros):
                groups = macro_groups(m)
                J0 = MACRO * m
                FDm = cols - J0
                lchunks = [(c, min(FDm, c + CHUNK)) for c in range(0, FDm, CHUNK)]
                q = qpool.tile([ROWS, pw], f32, tag="q")
                sq_on_act = 0  # per-block square pattern: [DVE, ACT, ACT]
                for ti, g in enumerate(groups):
                    gt = pool.tile([ROWS, cols], f16, tag="g")
                    nc.vector._custom_dve(gwrap, out=gt[:, 0:FDm],
                                          in0=fb[:, J0:cols],
                                          s0=negfa[:, g:g + 1], s1=MAGIC)
                    y = ypool.tile([ROWS, pw], f32, tag="y")
                    for (c0, c1) in lchunks:
                        nc.tensor.matmul(y[:, c0:c1], cb, gt[:, c0:c1],
                                         start=True, stop=True)
                    sq = pool.tile([ROWS, cols], f16, tag="sq")
                    if sq_on_act:
                        nc.scalar.activation(sq[:, 0:FDm], y[:, 0:FDm],
                                             act.Square)
                    else:
                        nc.vector._custom_dve(pair_sq, out=sq[:, 0:FDm],
                                              in0=y[:, 0:FDm])
                    sq_on_act = 0 if ti == len(groups) - 1 else 1
                    ob = onesb[:, ROWS * ti:ROWS * (ti + 1)]
                    for (c0, c1) in lchunks:
                        nc.tensor.matmul(q[:, c0:c1], ob, sq[:, c0:c1],
                                         start=(ti == 0),
                                         stop=(ti == len(groups) - 1))
                # tail: kern = exp(-0.5*t - sigma*exp(0.5*t + ln(sigma)))
                t = pool.tile([ROWS, cols], f16, tag="t")
                nc.scalar.activation(t[:, 0:FDm], q[:, 0:FDm], act.Ln,
                                     bias=soft2)
                rtp = pool.tile([ROWS, cols], f16, tag="rtp")
                nc.scalar.activation(rtp[:, 0:FDm], t[:, 0:FDm], act.Exp,
                                     bias=lnsig, scale=0.5)
                w = pool.tile([ROWS, cols], f16, tag="w")
                nc.vector.scalar_tensor_tensor(w[:, 0:FDm], t[:, 0:FDm], -0.5,
                                               rtp[:, 0:FDm],
                                               alu.mult, alu.subtract)
                kern = pool.tile([ROWS, cols], f16, tag="kern")
                nc.scalar.activation(kern[:, 0:FDm], w[:, 0:FDm], act.Exp)
                # column sums into the global PSUM accumulator
                for (gc0, gc1) in ((J0, CHUNK), (max(CHUNK, J0), cols)):
                    if gc0 >= gc1:
                        continue
                    bankA = gc0 < CHUNK
                    nc.tensor.matmul(acc[0:1, gc0:gc1], srcst[:, m:m + 1],
                                     kern[:, gc0 - J0:gc1 - J0],
                                     start=(m == 0),
                                     stop=(m == (bankA_stop if bankA
                                                 else n_macros - 1)))
                # row sums over beyond-macro columns (in-macro pairs are
                # already fully covered by the column sums)
                if FDm > MACRO:
                    kx = pool.tile([ROWS, cols], f16, tag="kx")
                    nc.vector._custom_dve(mulsum,
                                          out=kx[:, 0:FDm - MACRO],
                                          in0=kern[:, MACRO:FDm],
                                          in1=srcrep[:, J0 + MACRO:cols],
                                          accum_out=rowacc[:, m:m + 1])

            eo = pool.tile([1, cols], f32, tag="eo")
            for (c0, c1) in chunks:
                nc.scalar.copy(eo[0:1, c0:c1], acc[0:1, c0:c1])
                nc.sync.dma_start(OUT[0:1, c0:c1], eo[0:1, c0:c1])
            nc.sync.dma_start(OUTR[:], rowacc[:])
    nc.compile()
    return nc


def _get_program(n_macros, n_groups, cols, sigma, soft):
    key = (n_macros, n_groups, cols, round(sigma, 9), round(soft, 9))
    if key not in _cache:
        _cache[key] = _build(n_macros, n_groups, cols, sigma, soft)
    return _cache[key]


LAST_EXEC_TIME_NS = None


def kernel(pos, batch, cell, source, screening, softening, *, _trace=False):
    global LAST_EXEC_TIME_NS
    from concourse.bass_utils import run_bass_kernel_spmd

    pos = np.asarray(pos)
    cell = np.asarray(cell)
    source = np.asarray(source, dtype=np.float32)
    sigma = float(np.asarray(screening, dtype=np.float32))
    soft = float(np.asarray(softening, dtype=np.float32))

    n = pos.shape[0]
    nb = cell.shape[0]
    bi = np.asarray(batch).astype(np.int64)
    counts = np.bincount(bi, minlength=nb)
    starts = np.concatenate([[0], np.cumsum(counts)])
    assert nb == NCORES and np.all(np.diff(bi) >= 0)

    # host precompute in float64; fs = frac - 0.5 halves the fp16 repr error
    inv = np.linalg.inv(cell.astype(np.float64))
    fs = np.empty((n, 3), dtype=np.float64)
    for g in range(nb):
        i0, i1 = starts[g], starts[g + 1]
        fs[i0:i1] = pos[i0:i1].astype(np.float64) @ inv[g] - 0.5
    fs16 = fs.astype(np.float16)

    namax = int(counts.max())
    cols = -(-namax // 8) * 8          # padded atom count per core
    n_groups = -(-namax // GA)
    n_macros = -(-namax // MACRO)
    diag_c = float(np.exp(-np.float64(sigma) * np.float64(soft)) / np.float64(soft))

    idx_atom = np.arange(3 * GA) // 3
    idx_k = np.arange(3 * GA) % 3

    in_maps = []
    for g in range(nb):
        i0, i1 = starts[g], starts[g + 1]
        ng = i1 - i0
        fpad = np.zeros((cols, 3), dtype=np.float16)
        fpad[:ng] = fs16[i0:i1]

        fb = np.zeros((ROWS, cols), dtype=np.float16)
        fb[:3 * GA] = np.tile(fpad.T, (GA, 1))
        negfa = np.zeros((ROWS, n_groups), dtype=np.float32)
        for t in range(n_groups):
            a = np.minimum(t * GA + idx_atom, cols - 1)
            negfa[:3 * GA, t] = -fpad[a, idx_k].astype(np.float32)
        C = cell[g].astype(np.float16)
        cbm = np.zeros((ROWS, ROWS), dtype=np.float16)
        for i in range(GA):
            cbm[3 * i:3 * i + 3, 3 * i:3 * i + 3] = C
        onesb = np.zeros((ROWS, GPM, ROWS), dtype=np.float16)
        for t in range(GPM):
            for i in range(GA):
                onesb[3 * i:3 * i + 3, t, GA * t + i] = 1.0
        onesb = np.ascontiguousarray(onesb.reshape(ROWS, GPM * ROWS))
        spad = np.zeros(MACRO * n_macros, dtype=np.float16)
        spad[:ng] = source[i0:i1].astype(np.float16)
        srcst = np.zeros((ROWS, n_macros), dtype=np.float16)
        srcst[:MACRO] = spad.reshape(n_macros, MACRO).T
        srcrep = np.broadcast_to(spad[:cols].astype(np.float16)[None, :],
                                 (ROWS, cols)).copy()
        bloba = np.concatenate(
            [fb, negfa.view(np.float16).reshape(ROWS, -1)], axis=1)
        blobb = np.concatenate([cbm, onesb, srcst, srcrep], axis=1)
        in_maps.append({"BLOBA": np.ascontiguousarray(bloba),
                        "BLOBB": np.ascontiguousarray(blobb)})

    nc = _get_program(n_macros, n_groups, cols, sigma, soft)
    res = run_bass_kernel_spmd(nc, in_maps, list(range(NCORES)), trace=_trace)
    LAST_EXEC_TIME_NS = res.exec_time_ns

    out = np.zeros((n, 1), dtype=np.float32)
    for g in range(nb):
        i0, i1 = starts[g], starts[g + 1]
        ng = i1 - i0
        accg = res.results[g]["OUT"][0, :ng].astype(np.float64)
        rowa = res.results[g]["OUTR"].astype(np.float64)  # [128, n_macros]
        for m in range(n_macros):
            if cols - MACRO * m <= MACRO:
                rowa[:, m] = 0.0  # block had no beyond-macro columns
        rowflat = rowa[:MACRO].T.reshape(-1)[:ng]         # atom a = 126*m + p
        s = source[i0:i1].astype(np.float64)
        out[i0:i1, 0] = (0.5 * s * (accg + rowflat)
                         - 0.5 * s * s * diag_c).astype(np.float32)
    return out


# revision 24
# speedup vs baseline: 1.0058x; 1.0058x over previous
"""Trainium2 Bass kernel: dense screened-Coulomb pair energy with periodic
minimum-image convention (N=6144 atoms, B=8 cells), row-summed per atom.

batch is sorted and cross-graph pairs are masked, so the N x N problem is
block-diagonal over the 8 graphs: one graph per NeuronCore.

Math (fractional coords, fs = frac - 0.5):
  f_k[i,j] = fs_k[j] - fs_k[i]            DVE tensor_scalar fp16
  r_k      = round(f_k)                   DVE magic-number round fp16
  y        = (f - r) @ C                  PE: 2 accumulating fp16 matmuls
                                          (block-diag cell stationary, 128-wide
                                          weights for fast-weight-load)
  sq       = y^2                          ACT Square / custom DVE op (split)
  q        = sum_k y_k^2                  PE: ones-blockdiag fp16 matmul
  kern     = exp(-sigma*r)/r, r=sqrt(q+soft^2)
           = exp(-0.5*t - sigma*exp(0.5*t + ln(sigma)))  with t = ln(q+soft^2)
                                          ACT Ln, ACT Exp, DVE stt, ACT Exp
                                          -- Ln/Exp/Square served by ONE table
                                          set (natural_log_exp_and_others): no
                                          ACT_TABLE_LOAD thrash, no reciprocal.
  acc[j]   = sum_i src_i * kern[i,j]      PE matvec, PSUM-resident accumulator
                                          (row sum == col sum by symmetry)
  host: E[j] = 0.5*src_j*acc_j - 0.5*src_j^2*exp(-sigma*soft)/soft

Atoms in groups of GA=42 (3 coord rows per atom = 126 of 128 partitions);
3 groups form a 126-atom macro. All tiles padded to 128 partitions; padded
weight columns are zero, so padded q/kern rows compute to harmless zeros and
are masked by zero entries in the src weights.
"""
import numpy as np

GA = 42            # atoms per row group
ROWS = 128         # partitions per tile (3*GA = 126 used)
GPM = 3            # groups per macro block
MACRO = GA * GPM   # 126 atoms per macro
MAGIC = 12582912.0  # 1.5 * 2**23: (x + MAGIC) - MAGIC == round(x) for |x| < 2**22
NCORES = 8
CHUNK = 512        # PSUM bank limit for one matmul output (fp32 values)

_cache = {}
_custom_ops = None


def _register_custom(name, spec):
    import concourse.dve_ops as dve_ops
    from concourse.dve_spec import lower, _has_src1
    from concourse.dve_uop import DveOpSpec

    opcode = dve_ops._CUSTOM_DVE_ROW_BASE + len(dve_ops.OPS)
    shas = {}
    for ver in ("v3", "v4"):
        tmp = DveOpSpec(name=name, opcode=opcode,
                        uops=lower(spec, ver=ver), rd1_en=_has_src1(spec))
        shas[ver] = tmp.sha(ver)
    op = dve_ops.DveOp(name, spec, subdim=False, uops_sha=shas)
    dve_ops.OPS.append(op)
    dve_ops.CUSTOM_DVE_SPECS[op.name] = op.spec
    dve_ops._SUB_OPCODE_FOR_NAME[op.name] = opcode
    return op


def _get_custom_ops():
    """Register (once) two custom DVE ops:
    PAIR_SQ: out = in0^2 (single source; PSUM-legal square for the DVE share
             of the y->sq work -- stock tensor_tensor(y, y) would need two
             PSUM streams, which the STT struct forbids).
    GWRAP:   out = f - round(f) with f = in0 + s0, round via the fp32
             magic-number trick. One op replaces the f and r tensor_scalars
             and halves the y matmuls (single stationary weight cb)."""
    global _custom_ops
    if _custom_ops is None:
        from concourse.dve_spec import Spec, Src0, Src1, C0, C1, sq, AluOp

        sq_spec = Spec(body=sq(Src0),
                       reference=lambda in0, in1, s0, s1, imm2: in0 * in0)
        f = Src0 + C0
        g_spec = Spec(body=f - ((f + C1) - C1),
                      reference=lambda in0, in1, s0, s1, imm2:
                      (in0 + s0) - (((in0 + s0) + s1) - s1))
        ms_spec = Spec(body=Src0 * Src1, accum=AluOp.ADD,
                       reference=lambda in0, in1, s0, s1, imm2: in0 * in1)
        _custom_ops = (_register_custom("PAIR_SQ", sq_spec),
                       _register_custom("GWRAP", g_spec),
                       _register_custom("MULSUM", ms_spec))
    return _custom_ops


def _pin_lnexp_table(nc, mybir):
    """Bias the activation-table-load pass so Ln and Exp both resolve to the
    natural_log_exp_and_others set (the only set containing both): drop Ln
    from natural_log and Exp from exp_and_others in the (cached) table dict.
    Square stays available in every set, so the whole kernel needs ONE
    ACT_TABLE_LOAD instead of two per macro."""
    from concourse.hw_specs import get_activation_tables
    tabs = get_activation_tables(nc.m.arch)
    AF = mybir.ActivationFunctionType
    if "natural_log_exp_and_others" in tabs:
        for name, fns in tabs.items():
            if name != "natural_log_exp_and_others":
                fns.discard(AF.Ln)
                fns.discard(AF.Exp)
                fns.discard(AF.Square)
                fns.discard(AF.Copy)


def _build(n_macros, n_groups, cols, sigma, soft):
    import concourse.bacc as bacc
    import concourse.mybir as mybir
    import concourse.tile as tile

    f32 = mybir.dt.float32
    f16 = mybir.dt.float16
    alu = mybir.AluOpType
    act = mybir.ActivationFunctionType
    pair_sq, gwrap, mulsum = _get_custom_ops()

    soft2 = float(np.float32(soft) * np.float32(soft))
    lnsig = float(np.log(np.float64(sigma)))
    chunks = [(c, min(cols, c + CHUNK)) for c in range(0, cols, CHUNK)]
    pw = -(-cols // CHUNK) * CHUNK

    nc = bacc.Bacc("TRN2", target_bir_lowering=False, debug=False)
    _pin_lnexp_table(nc, mybir)
    # const APs for activation biases, registered like the built-ins
    for name, val in (("c-soft2", soft2), ("c-lnsig", lnsig)):
        t = nc.alloc_sbuf_tensor(name, [128, 1], f32)
        nc.gpsimd.memset(t.ap(), val)
        nc.const_aps.aps[(f32, val)] = t.ap()

    blob_a_w = cols + 2 * n_groups          # f16 units: fb | negfa(f32)
    blob_b_w = ROWS + GPM * ROWS + n_macros + cols
    BLOBA = nc.declare_dram_parameter("BLOBA", [ROWS, blob_a_w], f16, isOutput=False)
    BLOBB = nc.declare_dram_parameter("BLOBB", [ROWS, blob_b_w], f16, isOutput=False)
    OUT = nc.declare_dram_parameter("OUT", [1, cols], f32, isOutput=True)
    OUTR = nc.declare_dram_parameter("OUTR", [ROWS, n_macros], f32, isOutput=True)

    # group g -> (macro, slot); last macro may have < GPM groups
    def macro_groups(m):
        return [g for g in range(GPM * m, min(GPM * (m + 1), n_groups))]

    with tile.TileContext(nc) as tc:
        with tc.tile_pool(name="const", bufs=1) as cpool, \
             tc.tile_pool(name="work", bufs=6) as pool, \
             tc.tile_pool(name="ypsum", bufs=2, space="PSUM") as ypool, \
             tc.tile_pool(name="qpsum", bufs=1, space="PSUM") as qpool, \
             tc.tile_pool(name="apsum", bufs=1, space="PSUM") as apool:
            bloba = cpool.tile([ROWS, blob_a_w], f16)
            blobb = cpool.tile([ROWS, blob_b_w], f16)
            rowacc = cpool.tile([ROWS, n_macros], f32)
            nc.sync.dma_start(bloba[:], BLOBA[:])
            nc.gpsimd.dma_start(blobb[:], BLOBB[:])
            fb = bloba[:, 0:cols]
            negfa = bloba[:, cols:cols + 2 * n_groups].bitcast(f32)
            cb = blobb[:, 0:ROWS]
            onesb = blobb[:, ROWS:ROWS + GPM * ROWS]
            srcst = blobb[:, ROWS + GPM * ROWS:ROWS + GPM * ROWS + n_macros]
            srcrep = blobb[:, ROWS + GPM * ROWS + n_macros:blob_b_w]

            acc = apool.tile([1, pw], f32)

            # triangle over macro-row blocks: block m covers row atoms of
            # macro m x global columns [126m, cols). Column sums accumulate
            # in PSUM via PE; row sums (columns beyond the own macro, to
            # avoid double-count) via tensor_tensor_reduce with a
            # partition-replicated src tile.
            bankA_stop = max(m for m in range(n_macros) if MACRO * m < CHUNK)

            def emit_block_compute(m):
                """groups + tail for block m; returns the kern tile."""
                groups = macro_groups(m)
                J0 = MACRO * m
                FDm = cols - J0
                lchunks = [(c, min(FDm, c + CHUNK)) for c in range(0, FDm, CHUNK)]
                q = qpool.tile([ROWS, pw], f32, tag="q")
                sq_on_act = 0  # per-block square pattern: [DVE, ACT, ACT]
                for ti, g in enumerate(groups):
                    gt = pool.tile([ROWS, cols], f16, tag="g")
                    nc.vector._custom_dve(gwrap, out=gt[:, 0:FDm],
                                          in0=fb[:, J0:cols],
                                          s0=negfa[:, g:g + 1], s1=MAGIC)
                    y = ypool.tile([ROWS, pw], f32, tag="y")
                    for (c0, c1) in lchunks:
                        nc.tensor.matmul(y[:, c0:c1], cb, gt[:, c0:c1],
                                         start=True, stop=True)
                    sq = pool.tile([ROWS, cols], f16, tag="sq")
                    if sq_on_act:
                        nc.scalar.activation(sq[:, 0:FDm], y[:, 0:FDm],
                                             act.Square)
                    else:
                        nc.vector._custom_dve(pair_sq, out=sq[:, 0:FDm],
                                              in0=y[:, 0:FDm])
                    sq_on_act = 0 if ti == len(groups) - 1 else 1
                    ob = onesb[:, ROWS * ti:ROWS * (ti + 1)]
                    for (c0, c1) in lchunks:
                        nc.tensor.matmul(q[:, c0:c1], ob, sq[:, c0:c1],
                                         start=(ti == 0),
                                         stop=(ti == len(groups) - 1))
                t = pool.tile([ROWS, cols], f16, tag="t")
                nc.scalar.activation(t[:, 0:FDm], q[:, 0:FDm], act.Ln,
                                     bias=soft2)
                rtp = pool.tile([ROWS, cols], f16, tag="rtp")
                nc.scalar.activation(rtp[:, 0:FDm], t[:, 0:FDm], act.Exp,
                                     bias=lnsig, scale=0.5)
                w = pool.tile([ROWS, cols], f16, tag="w")
                nc.vector.scalar_tensor_tensor(w[:, 0:FDm], t[:, 0:FDm], -0.5,
                                               rtp[:, 0:FDm],
                                               alu.mult, alu.subtract)
                ktag = "kern_last" if m == n_macros - 1 else "kern"
                kern = pool.tile([ROWS, cols], f16, tag=ktag)
                nc.scalar.activation(kern[:, 0:FDm], w[:, 0:FDm], act.Exp)
                if FDm > MACRO:
                    kx = pool.tile([ROWS, cols], f16, tag="kx")
                    nc.vector._custom_dve(mulsum,
                                          out=kx[:, 0:FDm - MACRO],
                                          in0=kern[:, MACRO:FDm],
                                          in1=srcrep[:, J0 + MACRO:cols],
                                          accum_out=rowacc[:, m:m + 1])
                return kern

            def emit_block_acc(m, kern):
                J0 = MACRO * m
                for (gc0, gc1) in ((J0, CHUNK), (max(CHUNK, J0), cols)):
                    if gc0 >= gc1:
                        continue
                    bankA = gc0 < CHUNK
                    nc.tensor.matmul(acc[0:1, gc0:gc1], srcst[:, m:m + 1],
                                     kern[:, gc0 - J0:gc1 - J0],
                                     start=(m == 0),
                                     stop=(m == (bankA_stop if bankA
                                                 else n_macros - 1)))

            # the last (shortest) block's compute runs first, absorbing the
            # input-DMA wait; its acc-matmul stays last so the PSUM
            # accumulation flags keep their original order, and the final
            # chain shrinks to one tiny matmul + copy.
            kern_last = None
            if n_macros > 1 and MACRO * (n_macros - 1) >= CHUNK:
                kern_last = emit_block_compute(n_macros - 1)
            for m in range(n_macros - (1 if kern_last is not None else 0)):
                kern = emit_block_compute(m)
                emit_block_acc(m, kern)
            if kern_last is not None:
                emit_block_acc(n_macros - 1, kern_last)

            eo = pool.tile([1, cols], f32, tag="eo")
            for (c0, c1) in chunks:
                nc.scalar.copy(eo[0:1, c0:c1], acc[0:1, c0:c1])
                nc.sync.dma_start(OUT[0:1, c0:c1], eo[0:1, c0:c1])
            nc.sync.dma_start(OUTR[:], rowacc[:])
    nc.compile()
    return nc


def _get_program(n_macros, n_groups, cols, sigma, soft):
    key = (n_macros, n_groups, cols, round(sigma, 9), round(soft, 9))
    if key not in _cache:
        _cache[key] = _build(n_macros, n_groups, cols, sigma, soft)
    return _cache[key]


LAST_EXEC_TIME_NS = None


def kernel(pos, batch, cell, source, screening, softening, *, _trace=False):
    global LAST_EXEC_TIME_NS
    from concourse.bass_utils import run_bass_kernel_spmd

    pos = np.asarray(pos)
    cell = np.asarray(cell)
    source = np.asarray(source, dtype=np.float32)
    sigma = float(np.asarray(screening, dtype=np.float32))
    soft = float(np.asarray(softening, dtype=np.float32))

    n = pos.shape[0]
    nb = cell.shape[0]
    bi = np.asarray(batch).astype(np.int64)
    counts = np.bincount(bi, minlength=nb)
    starts = np.concatenate([[0], np.cumsum(counts)])
    assert nb == NCORES and np.all(np.diff(bi) >= 0)

    # host precompute in float64; fs = frac - 0.5 halves the fp16 repr error
    inv = np.linalg.inv(cell.astype(np.float64))
    fs = np.empty((n, 3), dtype=np.float64)
    for g in range(nb):
        i0, i1 = starts[g], starts[g + 1]
        fs[i0:i1] = pos[i0:i1].astype(np.float64) @ inv[g] - 0.5
    fs16 = fs.astype(np.float16)

    namax = int(counts.max())
    cols = -(-namax // 8) * 8          # padded atom count per core
    n_groups = -(-namax // GA)
    n_macros = -(-namax // MACRO)
    diag_c = float(np.exp(-np.float64(sigma) * np.float64(soft)) / np.float64(soft))

    idx_atom = np.arange(3 * GA) // 3
    idx_k = np.arange(3 * GA) % 3

    in_maps = []
    for g in range(nb):
        i0, i1 = starts[g], starts[g + 1]
        ng = i1 - i0
        fpad = np.zeros((cols, 3), dtype=np.float16)
        fpad[:ng] = fs16[i0:i1]

        fb = np.zeros((ROWS, cols), dtype=np.float16)
        fb[:3 * GA] = np.tile(fpad.T, (GA, 1))
        negfa = np.zeros((ROWS, n_groups), dtype=np.float32)
        for t in range(n_groups):
            a = np.minimum(t * GA + idx_atom, cols - 1)
            negfa[:3 * GA, t] = -fpad[a, idx_k].astype(np.float32)
        C = cell[g].astype(np.float16)
        cbm = np.zeros((ROWS, ROWS), dtype=np.float16)
        for i in range(GA):
            cbm[3 * i:3 * i + 3, 3 * i:3 * i + 3] = C
        onesb = np.zeros((ROWS, GPM, ROWS), dtype=np.float16)
        for t in range(GPM):
            for i in range(GA):
                onesb[3 * i:3 * i + 3, t, GA * t + i] = 1.0
        onesb = np.ascontiguousarray(onesb.reshape(ROWS, GPM * ROWS))
        spad = np.zeros(MACRO * n_macros, dtype=np.float16)
        spad[:ng] = source[i0:i1].astype(np.float16)
        srcst = np.zeros((ROWS, n_macros), dtype=np.float16)
        srcst[:MACRO] = spad.reshape(n_macros, MACRO).T
        srcrep = np.broadcast_to(spad[:cols].astype(np.float16)[None, :],
                                 (ROWS, cols)).copy()
        bloba = np.concatenate(
            [fb, negfa.view(np.float16).reshape(ROWS, -1)], axis=1)
        blobb = np.concatenate([cbm, onesb, srcst, srcrep], axis=1)
        in_maps.append({"BLOBA": np.ascontiguousarray(bloba),
                        "BLOBB": np.ascontiguousarray(blobb)})

    nc = _get_program(n_macros, n_groups, cols, sigma, soft)
    res = run_bass_kernel_spmd(nc, in_maps, list(range(NCORES)), trace=_trace)
    LAST_EXEC_TIME_NS = res.exec_time_ns

    out = np.zeros((n, 1), dtype=np.float32)
    for g in range(nb):
        i0, i1 = starts[g], starts[g + 1]
        ng = i1 - i0
        accg = res.results[g]["OUT"][0, :ng].astype(np.float64)
        rowa = res.results[g]["OUTR"].astype(np.float64)  # [128, n_macros]
        for m in range(n_macros):
            if cols - MACRO * m <= MACRO:
                rowa[:, m] = 0.0  # block had no beyond-macro columns
        rowflat = rowa[:MACRO].T.reshape(-1)[:ng]         # atom a = 126*m + p
        s = source[i0:i1].astype(np.float64)
        out[i0:i1, 0] = (0.5 * s * (accg + rowflat)
                         - 0.5 * s * s * diag_c).astype(np.float32)
    return out
